# revision 19
# baseline (speedup 1.0000x reference)
"""Trainium2 Bass kernel for nn_AttentiveGatingv2 (moe_routing).

Reference computation (shapes hardcoded):
  x: [64, 96, 207, 64] -> take last 6 timesteps -> per-(b,n) token:
  z = proj(x_k); qkv = in_proj(z); 4-head attention over the 6 steps;
  out-proj; mean over steps; fc to 8 experts; softmax -> [64, 207, 8].

Host-side algebraic fusion (verified vs reference):
  W_eff = in_proj_w @ proj_w  (96x64), b_eff = in_proj_w@proj_b + in_proj_b
  (q-rows pre-scaled by 1/sqrt(8)); since mean-over-steps commutes with the
  linear out-proj/fc, post-attention collapses to
  logits = G @ (sum_j wbar_j * v_j) + g_b  with  G = fc_w@out_w/6,
  g_b = fc_w@out_b + fc_b,  wbar_j = sum_i softmax_j(scores)_ij.

Layout strategy: host pre-slices the 6 needed timesteps (1/16 of x), casts
to bf16, packs them feature-major with an appended ones-row so the single
PE matmul  qkv[tokens,96] = x_aug[65,tokens].T @ W_aug[65,96]  lands
token-major in PSUM (fp32) with bias included.  Attention math runs on
VectorE in bf16 (validated ~5e-4 rel-to-max on the final softmax output),
with 128-token tiles processed in groups of 4 so the small softmax/context
ops amortize instruction overhead; expert-logit matmuls run per pair of
tiles through one PE transpose + a block-diagonal G.  8 NeuronCores
data-parallel over batch; no cross-device communication.

Measured on trn2 (8 cores, via axon): HW exec ~58.5us/core, rel err 5.4e-4.
Progression: 160us (first correct fp32) -> 90 (drop serializing hacks) ->
73 (bf16) -> 62 (pair batching) -> 58.5 (quad batching, load stagger,
scalar-queue const loads, deeper work pool). GQ=6 measured
worse (60.4us: batched ops stall behind 6 evacuations), so GQ=4 stands.
"""

import numpy as np
import ml_dtypes

import concourse.bass as bass
import concourse.mybir as mybir
import concourse.tile as tile
from concourse.bacc import Bacc
from concourse.bass_utils import run_bass_kernel_spmd

F32 = mybir.dt.float32
BF16 = mybir.dt.bfloat16
NP_BF16 = ml_dtypes.bfloat16

# problem dims
B, T, NTOK, C = 64, 96, 207, 64
D, H, HD, K = 32, 4, 8, 6
E = 8
NCORES = 8

# per-core dims
B_SH = B // NCORES            # 8
S = B_SH * NTOK               # 1656 tokens per core
P = 128
NT = (S + P - 1) // P         # 13 tiles
S_PAD = NT * P                # 1664
CA = C + 1                    # 65: channels + ones row
E3 = 3 * D                    # 96
KK = K * K                    # 36
DA = D + 1                    # 33


def _build_module():
    nc = Bacc()

    xt = nc.dram_tensor("xt", [CA, K, S_PAD], BF16, kind="ExternalInput")
    # wa columns: q (32, pre-scaled) | k (32) | ve (32 = 4 heads x 8 experts)
    # where ve_(h,e) = G[e, 8h:8h+8] @ v_(h,:) + g_b[e]/24, so
    # logits = sum_{j,h} wbar[j,h] * ve[j,h,:] exactly (sum wbar = 24).
    wa = nc.dram_tensor("wa", [CA, E3], BF16, kind="ExternalInput")
    # out[p, t, e]: token (t*128+p); host reassembles. This layout keeps the
    # single final store one-descriptor-per-partition contiguous.
    out = nc.dram_tensor("out", [P, NT, E], F32, kind="ExternalOutput")

    AF = mybir.ActivationFunctionType
    AX = mybir.AxisListType

    def apv(t, dims, extra_offset=0):
        # custom AP over tile t: keep t's partition dim, replace free dims
        return bass.AP(
            tensor=t.tensor,
            offset=t.offset + extra_offset,
            ap=[list(t.ap[0])] + [list(d) for d in dims],
        )

    GQ = 4                                                   # tiles per group
    groups = [(t, min(GQ, NT - t)) for t in range(0, NT, GQ)]  # (base, size)

    with tile.TileContext(nc) as tc:
        with (
            tc.tile_pool(name="singles", bufs=1) as singles,
            tc.tile_pool(name="xload", bufs=4) as xload,
            tc.tile_pool(name="work", bufs=3) as work,
            tc.tile_pool(name="psum", bufs=2, space="PSUM") as psum,
        ):
            # DMA issue costs ~0.8us each on a sequencer: put the two
            # constant loads on the Scalar HWDGE queue (issues in parallel
            # with the Sync queue issuing x loads), and load x per tile-PAIR
            # to halve the issue count.
            wa_sb = singles.tile([CA, E3], BF16)
            nc.scalar.dma_start(out=wa_sb, in_=wa[:, :])

            # Split the first group's loads per-tile so tile 0 lands ASAP
            # (kills the ~7us vector startup stall waiting on a 400KB load).
            xg_tiles = []
            xg_dmas = []
            for gi, (tg, g) in enumerate(groups):
                xg_sb = xload.tile([CA, K, GQ * P], BF16, name=f"xg_sb{gi}",
                                   tag="xg")
                if gi == 0:
                    xds = []
                    for u in range(g):
                        xds.append(nc.sync.dma_start(
                            out=xg_sb[:, :, u * P:(u + 1) * P],
                            in_=xt[:, :, (tg + u) * P:(tg + u + 1) * P]))
                    xd = xds[0]
                else:
                    xd = nc.sync.dma_start(
                        out=xg_sb[:, :, 0:g * P],
                        in_=xt[:, :, tg * P:(tg + g) * P])
                xg_tiles.append(xg_sb)
                xg_dmas.append(xd)

            out_sb = singles.tile([P, NT, E], F32)
            ltr_all = singles.tile([P, NT, E], F32)
            es_all = singles.tile([P, NT, K, K, H], BF16)
            ve_all = singles.tile([P, NT, K, D], BF16)

            qkv_first = []
            for gi, (tg, g) in enumerate(groups):
                if gi >= 2:
                    # stagger loads ~2 groups ahead of compute so early tiles
                    # don't round-robin behind all the loads
                    tile.add_dep_helper(xg_dmas[gi].ins,
                                        qkv_first[gi - 2].ins,
                                        sync=True, reason="load stagger")
                qk_sb = work.tile([P, GQ, K, 2 * D], BF16)
                tmp = work.tile([P, GQ, KK, D], BF16)
                for u in range(g):
                    t = tg + u
                    xt_sb = xg_tiles[gi][:, :, u * P:(u + 1) * P]

                    # ---- q|k|ve: 6 matmuls (bf16 in, fp32 psum) ----
                    # [P, 8, 128] = exactly 2 PSUM banks so slots are
                    # bank-aligned (1.5-bank slots would share a bank)
                    qkv_ps = psum.tile([P, 8, 128], F32, tag="qkv_ps", bufs=2,
                                       name="qkv_ps")
                    for i in range(K):
                        mm = nc.tensor.matmul(
                            out=qkv_ps[:, i, 0:E3],
                            lhsT=xt_sb[:, i, :],
                            rhs=wa_sb[:, :],
                            start=True,
                            stop=True,
                        )
                        if i == 0 and u == 0:
                            qkv_first.append(mm)

                    # ---- evacuate to SBUF as bf16 on ScalarE; ve goes to
                    # its own persistent tile so (j,h) is stride-8 mergeable
                    # and phase B can read all 13 tiles at once ----
                    nc.scalar.copy(out=qk_sb[:, u], in_=qkv_ps[:, 0:K, 0:2 * D])
                    nc.scalar.copy(out=ve_all[:, t],
                                   in_=qkv_ps[:, 0:K, 2 * D:E3])

                    # ---- scores tmp[i,j,(h,c)] = q[i,(hc)] * k[j,(hc)] ----
                    # (TensorTensor allows at most 3 free AP dims: (h,c) is
                    # kept merged, and the i/j broadcasts force per-tile muls)
                    off = u * K * 2 * D
                    q_ap = apv(qk_sb, [[2 * D, K], [0, K], [1, D]], off)
                    k_ap = apv(qk_sb, [[0, K], [2 * D, K], [1, D]], off + D)
                    tm_out = apv(tmp, [[D, KK], [1, D]], u * KK * D)
                    nc.vector.tensor_mul(tm_out, q_ap, k_ap)

                # ---- batched over the pair from here on ----
                gKK = g * KK
                # scores[(t,i,j), h] = sum_c tmp via add tree: tensor_reduce
                # costs input-elems (2304/pair @1x) vs 1152+576+576 for the
                # tree (TT cost follows output elems)
                s1 = work.tile([P, GQ, KK, H, 4], BF16)
                a_ap = apv(tmp, [[D, gKK], [HD, H], [1, 4]])
                b_ap = apv(tmp, [[D, gKK], [HD, H], [1, 4]], 4)
                o_ap = apv(s1, [[16, gKK], [4, H], [1, 4]])
                nc.vector.tensor_add(o_ap, a_ap, b_ap)
                s2 = work.tile([P, GQ, KK, H, 2], BF16)
                a_ap = apv(s1, [[16, gKK], [4, H], [1, 2]])
                b_ap = apv(s1, [[16, gKK], [4, H], [1, 2]], 2)
                o_ap = apv(s2, [[8, gKK], [2, H], [1, 2]])
                nc.vector.tensor_add(o_ap, a_ap, b_ap)
                scores = work.tile([P, GQ, KK, H], F32)
                a_ap = apv(s2, [[8, gKK], [2, H]])
                b_ap = apv(s2, [[8, gKK], [2, H]], 1)
                o_ap = apv(scores, [[H, gKK], [1, H]])
                nc.vector.tensor_add(o_ap, a_ap, b_ap)

                # ---- exp into the persistent all-tiles buffer ----
                nc.scalar.activation(out=es_all[:, tg:tg + g],
                                     in_=scores[:, 0:g], func=AF.Exp)

            # ==== phase B: softmax-normalize + logits, batched over all
            # 13 tiles (big ops amortize the 58-120cyc DVE fixed cost and
            # cut cross-engine semaphore traffic) ====
            NTK = NT * K
            KH = K * H
            # Z_i: add-tree over j (6 = 3+3)
            zs1 = singles.tile([P, NT, K, 3, H], BF16)
            nc.vector.tensor_add(
                apv(zs1, [[3 * H, NTK], [H, 3], [1, H]]),
                apv(es_all, [[K * H, NTK], [H, 3], [1, H]]),
                apv(es_all, [[K * H, NTK], [H, 3], [1, H]], 3 * H))
            zs2 = singles.tile([P, NT, K, H], BF16)
            nc.vector.tensor_add(
                apv(zs2, [[H, NTK], [1, H]]),
                apv(zs1, [[3 * H, NTK], [1, H]]),
                apv(zs1, [[3 * H, NTK], [1, H]], H))
            zs = singles.tile([P, NT, K, H], F32)
            nc.vector.tensor_add(
                apv(zs, [[H, NTK], [1, H]]),
                apv(zs2, [[H, NTK], [1, H]]),
                apv(zs1, [[3 * H, NTK], [1, H]], 2 * H))
            rs = singles.tile([P, NT, K, H], F32)
            nc.vector.reciprocal_approx_fast(
                apv(rs, [[1, NTK * H]]), apv(zs, [[1, NTK * H]]))
            # bf16 copy of 1/Z so the attn mul is all-bf16 (2x eligible)
            rsb = singles.tile([P, NT, K, H], BF16)
            nc.vector.tensor_copy(
                apv(rsb, [[1, NTK * H]]), apv(rs, [[1, NTK * H]]))
            # attn[(t,i),j,h] = es * rs
            attn = singles.tile([P, NT, K, K, H], BF16)
            nc.vector.tensor_mul(
                apv(attn, [[K * H, NTK], [H, K], [1, H]]),
                apv(es_all, [[K * H, NTK], [H, K], [1, H]]),
                apv(rsb, [[H, NTK], [0, K], [1, H]]))
            # wbar[t,(j,h)] = sum_i attn: tree over i (6 = 3+3)
            w1 = singles.tile([P, NT, 3, K, H], BF16)
            nc.vector.tensor_add(
                apv(w1, [[3 * KH, NT], [KH, 3], [1, KH]]),
                apv(attn, [[K * KH, NT], [KH, 3], [1, KH]]),
                apv(attn, [[K * KH, NT], [KH, 3], [1, KH]], 3 * KH))
            w2 = singles.tile([P, NT, K, H], BF16)
            nc.vector.tensor_add(
                apv(w2, [[KH, NT], [1, KH]]),
                apv(w1, [[3 * KH, NT], [1, KH]]),
                apv(w1, [[3 * KH, NT], [1, KH]], KH))
            wbar = singles.tile([P, NT, K, H], BF16)
            nc.vector.tensor_add(
                apv(wbar, [[KH, NT], [1, KH]]),
                apv(w2, [[KH, NT], [1, KH]]),
                apv(w1, [[3 * KH, NT], [1, KH]], 2 * KH))
            # broadcast wbar over e via copy, then fully-contiguous lt mul
            wbe = singles.tile([P, NT, KH, E], BF16)
            nc.vector.tensor_copy(
                apv(wbe, [[KH * E, NT], [E, KH], [1, E]]),
                apv(wbar, [[KH, NT], [1, KH], [0, E]]))
            lt = singles.tile([P, NT, KH, E], BF16)
            nc.vector.tensor_mul(
                apv(lt, [[1, NT * KH * E]]),
                apv(wbe, [[1, NT * KH * E]]),
                apv(ve_all, [[1, NT * KH * E]]))
            # logits = sum_(jh) lt: tree 24 -> 12 -> 6 -> 3 -> 2+1
            lt1 = singles.tile([P, NT, 12, E], BF16)
            nc.vector.tensor_add(
                apv(lt1, [[96, NT], [1, 96]]),
                apv(lt, [[KH * E, NT], [1, 96]]),
                apv(lt, [[KH * E, NT], [1, 96]], 96))
            lt2 = singles.tile([P, NT, 6, E], BF16)
            nc.vector.tensor_add(
                apv(lt2, [[48, NT], [1, 48]]),
                apv(lt1, [[96, NT], [1, 48]]),
                apv(lt1, [[96, NT], [1, 48]], 48))
            lt3 = singles.tile([P, NT, 3, E], BF16)
            nc.vector.tensor_add(
                apv(lt3, [[24, NT], [1, 24]]),
                apv(lt2, [[48, NT], [1, 24]]),
                apv(lt2, [[48, NT], [1, 24]], 24))
            lt4 = singles.tile([P, NT, E], BF16)
            nc.vector.tensor_add(
                apv(lt4, [[E, NT], [1, E]]),
                apv(lt3, [[24, NT], [1, E]]),
                apv(lt3, [[24, NT], [1, E]], E))
            nc.vector.tensor_add(
                apv(ltr_all, [[E, NT], [1, E]]),
                apv(lt4, [[E, NT], [1, E]]),
                apv(lt3, [[24, NT], [1, E]], 2 * E))
            # final softmax over 8 experts + single store
            el = singles.tile([P, NT, E], F32)
            nc.scalar.activation(out=el, in_=ltr_all, func=AF.Exp)
            zf = singles.tile([P, NT], F32)
            nc.vector.reduce_sum(out=zf, in_=el, axis=AX.X)
            rf = singles.tile([P, NT], F32)
            nc.vector.reciprocal_approx_fast(rf, zf)
            rf_ap = apv(rf, [[1, NT], [0, E]])
            nc.vector.tensor_mul(out_sb, el, rf_ap)
            nc.gpsimd.dma_start(out=out[:, :, :], in_=out_sb)

    nc.finalize()
    return nc


_NC = None


def _get_module():
    global _NC
    if _NC is None:
        _NC = _build_module()
    return _NC


def _host_prep(x, proj_w, proj_b, in_proj_w, in_proj_b, out_w, out_b, fc_w, fc_b):
    scale = np.float32(1.0 / np.sqrt(HD))
    w_eff = (in_proj_w @ proj_w).astype(np.float32)          # [96, 64]
    b_eff = (in_proj_w @ proj_b + in_proj_b).astype(np.float32)
    w_eff[0:D] *= scale
    b_eff[0:D] *= scale

    g = (fc_w @ out_w / np.float32(K)).astype(np.float32)    # [8, 32]
    g_b = (fc_w @ out_b + fc_b).astype(np.float32)
    # fold out-proj+fc into per-(step,head) partial-logit weights:
    # ve_(h,e) = G[e, 8h:8h+8] @ v_(h,:) + g_b[e]/24; since
    # sum_{j,h} wbar[j,h] = K*H = 24, logits = sum_{j,h} wbar*ve exactly.
    for h in range(H):
        gh = g[:, h * HD:(h + 1) * HD]                       # [8, 8]
        rows = slice(2 * D + h * HD, 2 * D + (h + 1) * HD)
        w_eff[rows] = gh @ w_eff[rows]
        b_eff[rows] = gh @ b_eff[rows] + g_b / np.float32(K * H)

    wa = np.concatenate([w_eff.T, b_eff[None, :]], axis=0)   # [65, 96]
    wa = np.ascontiguousarray(wa).astype(NP_BF16)

    # x: [B, T, N, C] -> last K steps -> per-core [65, K, S_PAD] feature-major
    xk = x[:, T - K:, :, :]                                  # [B, K, N, C]
    in_maps = []
    for core in range(NCORES):
        xc = xk[core * B_SH:(core + 1) * B_SH]               # [8, K, N, C]
        # -> [C, K, b, N] -> [C, K, S]
        xc = np.transpose(xc, (3, 1, 0, 2)).reshape(C, K, S)
        xtc = np.ones((CA, K, S_PAD), dtype=NP_BF16)
        xtc[0:C, :, 0:S] = xc.astype(NP_BF16)
        xtc[0:C, :, S:] = 0
        in_maps.append({"xt": xtc, "wa": wa})
    return in_maps


def kernel(x, proj_w, proj_b, in_proj_w, in_proj_b, out_w, out_b, fc_w, fc_b,
           _trace=False):
    in_maps = _host_prep(np.asarray(x, dtype=np.float32),
                         np.asarray(proj_w, dtype=np.float32),
                         np.asarray(proj_b, dtype=np.float32),
                         np.asarray(in_proj_w, dtype=np.float32),
                         np.asarray(in_proj_b, dtype=np.float32),
                         np.asarray(out_w, dtype=np.float32),
                         np.asarray(out_b, dtype=np.float32),
                         np.asarray(fc_w, dtype=np.float32),
                         np.asarray(fc_b, dtype=np.float32))
    nc = _get_module()
    res = run_bass_kernel_spmd(nc, in_maps, core_ids=list(range(NCORES)),
                               trace=_trace)
    outs = []
    for core in range(NCORES):
        oc = res.results[core]["out"]                        # [P, NT, E]
        oc = oc.transpose(1, 0, 2).reshape(S_PAD, E)[:S]
        oc = oc.reshape(B_SH, NTOK, E)
        outs.append(oc)
    full = np.concatenate(outs, axis=0)                      # [64, 207, 8]
    if _trace:
        kernel._last_exec_time_ns = res.exec_time_ns
        kernel._last_profile = res.profile_json
    return full.astype(np.float32)

